# revision 34
# baseline (speedup 1.0000x reference)
"""Trainium2 Bass kernel for nn_Attention_15539191677265.

Single-head-dim attention block:
    qkv = w_qkv @ x ; per-head scaled dot-product attention over w=2048;
    out = w_out @ attn_out + b_out

Sharding: pure data-parallel over batch (b=8 -> 8 NeuronCores, one batch
element per core). Weights are replicated. No collectives.

Per-core algorithm (transposed-softmax, bf16 matmuls):
  1. q,k projections, then DMA-duplicate each head's 64 d-rows into both
     PE row halves so consecutive sim matmuls alternate row groups.
  2. vT = x.T @ wvT with 64 ones-columns appended: AV psum rows 64..127
     accumulate the softmax normalizer PRE-BROADCAST across partitions.
  3. per (head, i-half 1024, j-tile): sim^T strip as two full-width K=64
     matmuls on alternating PE row-halves (tile_position (0,0)/(64,0)) --
     disjoint rows + different psum banks stream concurrently (~2x column
     rate); exp as ONE [128,1024] op, split across engines: 9/16 j-tiles
     on ScalarE (exact exp), 7/16 on VectorE via the Schraudolph bf16
     bit-trick (x*128/ln2 + magic -> int16 bits == bf16 exp approx,
     ~1.8% rms weight noise, cancels through the shared normalizer);
     AV (M=128, K=128) accumulates over j-tiles in 2-jt batches lagging
     sim by 2-3 j-tiles so the PE never waits on the exp engines; each
     block's last two AVs + normalization are emitted inside the NEXT
     block's first j-tiles (cross-block pipelining) so the final exp
     wait is filled with the next block's sim matmuls. Three strip
     buffers + one AV buffer (8 psum banks exactly): sims wait on
     exp(jt-3), hiding the strip-rotation semaphore latency.
  4. normalize: copy av -> SBUF (ScalarE/VectorE), DMA-shift the
     normalizer rows to partitions 0..63, reciprocal (DVE custom op at
     base partition 0), multiply on GpSimd; odd heads bounce via DMA.
  5. proj: out = woutT.T @ attn_out (K=128 head-pair chunks) + bias.

No max subtraction before exp: scores are ~N(0,1) so exp cannot
overflow in bf16/fp32.
"""

import sys

if "/opt/trn_rl_repo" not in sys.path:
    sys.path.insert(0, "/opt/trn_rl_repo")

import numpy as np
import ml_dtypes

import concourse.bass as bass
import concourse.mybir as mybir
import concourse.tile as tile
from concourse import bacc
from concourse.bass_utils import run_bass_kernel_spmd

BF16 = mybir.dt.bfloat16
F32 = mybir.dt.float32
I16 = mybir.dt.int16
EXP = mybir.ActivationFunctionType.Exp

B, DIM, W = 8, 256, 2048
HEADS, DH = 8, 64
HID = HEADS * DH  # 512
SCALE = DH ** (-0.5)
N_CORES = 8

NJT = W // 128  # 16 j-tiles per head
NCT = DIM // 128  # 2 contraction chunks over channels
IH = 1024  # i-half width
NIH = W // IH

# j-tiles whose exp runs on VectorE via Schraudolph (rest: exact on ScalarE)
DVE_JTS = frozenset({1, 3, 5, 8, 10, 12, 14})
SCH_MUL = 128.0 / float(np.log(2.0))  # 184.6645
SCH_ADD = 127.0 * 128.0 - 7.4  # PWL-centering constant (round-to-nearest)


def build_kernel():
    nc = bacc.Bacc(None, target_bir_lowering=False)

    x_d = nc.dram_tensor("x", [DIM, W], BF16, kind="ExternalInput")
    wqkvT_d = nc.dram_tensor("wqkvT", [DIM, 3 * HID], BF16, kind="ExternalInput")
    woutT_d = nc.dram_tensor("woutT", [128, 4, DIM], BF16, kind="ExternalInput")
    bias_d = nc.dram_tensor("bias", [128, DIM // 128], F32, kind="ExternalInput")
    out_d = nc.dram_tensor("out", [DIM, W], F32, kind="ExternalOutput")

    with tile.TileContext(nc) as tc:
        with tc.tile_pool(name="pers", bufs=1) as pers:
            x_sb = pers.tile([128, NCT, W], BF16)
            wq_sb = pers.tile([128, NCT, 3 * HID], BF16)
            wo_sb = pers.tile([128, 4, DIM], BF16)
            bias_sb = pers.tile([128, DIM // 128], F32)
            # [128 = head d-rows duplicated in both halves, head, w]
            q_sb = pers.tile([128, HEADS, W], BF16)
            k_sb = pers.tile([128, HEADS, W], BF16)
            vt_sb = pers.tile([128, NJT, HEADS, 128], BF16)
            attout_sb = [
                pers.tile([128, W], BF16, name=f"attout_{kc}", tag=f"attout{kc}")
                for kc in range(4)
            ]
            out_sb = pers.tile([128, NCT, W], F32)

            xr = x_d[:].rearrange("(ct p) w -> p ct w", p=128)
            for ct in range(NCT):
                for wh in range(4):
                    nc.sync.dma_start(
                        out=x_sb[:, ct, wh * 512 : (wh + 1) * 512],
                        in_=xr[:, ct, wh * 512 : (wh + 1) * 512],
                    )
            wqr = wqkvT_d[:].rearrange("(ct p) o -> p ct o", p=128)
            for sec in (2, 0, 1):  # v weights first: v-proj unblocks earliest
                for ct in range(NCT):
                    nc.sync.dma_start(
                        out=wq_sb[:, ct, sec * HID : (sec + 1) * HID],
                        in_=wqr[:, ct, sec * HID : (sec + 1) * HID],
                    )
            nc.sync.dma_start(out=wo_sb[:], in_=woutT_d[:])
            nc.sync.dma_start(out=bias_sb[:], in_=bias_d[:])

            # vT cols 64..127: ALL ones -> AV rows 64..127 hold the softmax
            # normalizer replicated across 64 partitions (pre-broadcast)
            nc.vector.memset(vt_sb[:, :, :, DH:128], 1.0)
            # warm the ACT exp table while qkv matmuls run
            warm = pers.tile([1, 1], F32)
            nc.vector.memset(warm[:], 0.0)
            nc.scalar.activation(out=warm[:], in_=warm[:], func=EXP)

            # ---- phase 1: v first (attention needs all of vT), then q,k
            with tc.tile_pool(name="qkv_ps", bufs=4, space="PSUM") as qkv_ps:

                def emit_qk_otile(ot):
                    for dst, base in ((q_sb, 0), (k_sb, HID)):
                        for ph in range(4):
                            po = ph * 512
                            ps = qkv_ps.tile(
                                [128, 512], F32, name=f"qk_{ot}_{base}_{ph}", tag="qk"
                            )
                            for ct in range(NCT):
                                nc.tensor.matmul(
                                    ps[:],
                                    lhsT=wq_sb[:, ct, base + ot * 128 : base + (ot + 1) * 128],
                                    rhs=x_sb[:, ct, po : po + 512],
                                    start=(ct == 0),
                                    stop=(ct == NCT - 1),
                                )
                            if ph % 2 == 0:
                                nc.vector.tensor_copy(
                                    out=dst[0:64, 2 * ot, po : po + 512],
                                    in_=ps[0:64, :],
                                )
                                nc.scalar.copy(
                                    out=dst[64:128, 2 * ot + 1, po : po + 512],
                                    in_=ps[64:128, :],
                                )
                            else:
                                nc.scalar.copy(
                                    out=dst[0:64, 2 * ot, po : po + 512],
                                    in_=ps[0:64, :],
                                )
                                nc.vector.tensor_copy(
                                    out=dst[64:128, 2 * ot + 1, po : po + 512],
                                    in_=ps[64:128, :],
                                )
                        # partition-duplicating DMAs (overlap with matmuls)
                        nc.sync.dma_start(
                            out=dst[64:128, 2 * ot, :], in_=dst[0:64, 2 * ot, :]
                        )
                        nc.sync.dma_start(
                            out=dst[0:64, 2 * ot + 1, :], in_=dst[64:128, 2 * ot + 1, :]
                        )

                for jt in range(NJT):
                    ps = qkv_ps.tile([128, HID], F32, name=f"vt_{jt}", tag="vt")
                    for ct in range(NCT):
                        nc.tensor.matmul(
                            ps[:],
                            lhsT=x_sb[:, ct, jt * 128 : (jt + 1) * 128],
                            rhs=wq_sb[:, ct, 2 * HID : 3 * HID],
                            start=(ct == 0),
                            stop=(ct == NCT - 1),
                        )
                    if jt % 2 == 0:
                        nc.vector.tensor_copy(
                            out=vt_sb[:, jt, :, 0:DH],
                            in_=ps[:].rearrange("p (h d) -> p h d", h=HEADS),
                        )
                    else:
                        nc.scalar.copy(
                            out=vt_sb[:, jt, :, 0:DH],
                            in_=ps[:].rearrange("p (h d) -> p h d", h=HEADS),
                        )

                for ot in range(4):
                    emit_qk_otile(ot)

            # ---- phase 3: attention ----
            with (
                tc.tile_pool(name="strip_ps", bufs=3, space="PSUM") as strip_ps,
                tc.tile_pool(name="av_ps", bufs=1, space="PSUM") as av_ps,
                tc.tile_pool(name="exp_sb", bufs=6) as exp_pool,
                tc.tile_pool(name="norm_sb", bufs=2) as norm_pool,
            ):
                def emit_av(av, h, jt, es_tiles):
                    es = es_tiles[jt]
                    for c in range(IH // 512):
                        co = c * 512
                        nc.tensor.matmul(
                            av[:, co : co + 512],
                            lhsT=vt_sb[:, jt, h, :],
                            rhs=es[:, co : co + 512],
                            start=(jt == 0),
                            stop=(jt == NJT - 1),
                        )

                def norm_chain(av, h, io):
                    ncp = norm_pool.tile([128, IH], F32, tag="ncp")
                    if h % 2 == 0:
                        nc.scalar.copy(out=ncp[:], in_=av[:])
                    else:
                        nc.vector.tensor_copy(out=ncp[:], in_=av[:])
                    nsh = norm_pool.tile([DH, IH], F32, tag="nsh")
                    nc.sync.dma_start(out=nsh[:], in_=ncp[DH:128, :])
                    # custom-DVE recip requires base partition 0
                    nrm = norm_pool.tile([DH, IH], F32, tag="nrm")
                    nc.vector.reciprocal_approx_fast(out=nrm[:], in_=nsh[:])
                    # SBUF-only multiply on the otherwise-idle GpSimd
                    if h % 2 == 0:
                        nc.gpsimd.tensor_mul(
                            out=attout_sb[h // 2][0:DH, io : io + IH],
                            in0=ncp[0:DH, :],
                            in1=nrm[:],
                        )
                    else:
                        # odd heads land on partitions 64..127: bounce
                        atmp = norm_pool.tile([DH, IH], BF16, tag="atmp")
                        nc.gpsimd.tensor_mul(
                            out=atmp[:], in0=ncp[0:DH, :], in1=nrm[:]
                        )
                        nc.sync.dma_start(
                            out=attout_sb[h // 2][DH:128, io : io + IH],
                            in_=atmp[:],
                        )

                # odd heads (bounce DMA) early; cross-block pipelining: each
                # block's last two AVs + norm chain are emitted inside the
                # NEXT block's first j-tiles so the PE fills the final exp
                # wait with the next block's sim matmuls
                pending = None
                for h in (1, 3, 5, 7, 0, 2, 4, 6):
                    for ih in range(NIH):
                        io = ih * IH
                        av = av_ps.tile([128, IH], F32, name=f"av_{h}_{ih}", tag="av")
                        es_tiles = {}
                        for jt in range(NJT):
                            strip = strip_ps.tile(
                                [128, IH], F32, name=f"st_{h}_{ih}_{jt}", tag="st"
                            )
                            for c in range(IH // 512):
                                co = c * 512
                                # full-width K=64 matmuls on alternating PE
                                # row-halves: adjacent ops use disjoint rows
                                # and different psum banks -> concurrent
                                rg = 64 * (c % 2)
                                nc.tensor.matmul(
                                    strip[:, co : co + 512],
                                    lhsT=k_sb[rg : rg + 64, h, jt * 128 : (jt + 1) * 128],
                                    rhs=q_sb[rg : rg + 64, h, io + co : io + co + 512],
                                    start=True,
                                    stop=True,
                                    tile_position=(rg, 0),
                                )
                            es = exp_pool.tile([128, IH], BF16, name=f"es_{jt}", tag="es")
                            es_tiles[jt] = es
                            if jt in DVE_JTS:
                                nc.vector.tensor_scalar(
                                    out=es[:].bitcast(I16),
                                    in0=strip[:],
                                    scalar1=SCH_MUL,
                                    scalar2=SCH_ADD,
                                    op0=mybir.AluOpType.mult,
                                    op1=mybir.AluOpType.add,
                                )
                            else:
                                nc.scalar.activation(out=es[:], in_=strip[:], func=EXP)
                            if jt == 0:
                                if pending is not None:
                                    emit_av(pending[0], pending[1], NJT - 2, pending[3])
                            elif jt == 1:
                                if pending is not None:
                                    emit_av(pending[0], pending[1], NJT - 1, pending[3])
                                    norm_chain(pending[0], pending[1], pending[2])
                            elif jt % 2 == 1:
                                # AVs in 2-jt batches: halves the PE's
                                # 64-row/128-row tile-config switches
                                emit_av(av, h, jt - 3, es_tiles)
                                emit_av(av, h, jt - 2, es_tiles)
                        pending = (av, h, io, es_tiles)
                emit_av(pending[0], pending[1], NJT - 2, pending[3])
                emit_av(pending[0], pending[1], NJT - 1, pending[3])
                norm_chain(pending[0], pending[1], pending[2])

            # ---- phase 4: output projection + bias ----
            outr = out_d[:].rearrange("(ct p) w -> p ct w", p=128)
            with tc.tile_pool(name="proj_ps", bufs=8, space="PSUM") as proj_ps:
                for ot in range(NCT):
                    for wh in range(4):
                        wo = wh * 512
                        ps = proj_ps.tile([128, 512], F32, name=f"pj_{ot}_{wh}", tag="pj")
                        for kc in range(4):
                            nc.tensor.matmul(
                                ps[:],
                                lhsT=wo_sb[:, kc, ot * 128 : (ot + 1) * 128],
                                rhs=attout_sb[kc][:, wo : wo + 512],
                                start=(kc == 0),
                                stop=(kc == 3),
                            )
                        nc.vector.tensor_scalar_add(
                            out=out_sb[:, ot, wo : wo + 512],
                            in0=ps[:],
                            scalar1=bias_sb[:, ot : ot + 1],
                        )
                        nc.sync.dma_start(
                            out=outr[:, ot, wo : wo + 512],
                            in_=out_sb[:, ot, wo : wo + 512],
                        )

    nc.compile()
    return nc


_NC_CACHE = None


def _get_nc():
    global _NC_CACHE
    if _NC_CACHE is None:
        _NC_CACHE = build_kernel()
    return _NC_CACHE


def make_in_maps(x, w_qkv, w_out, b_out):
    bf16 = ml_dtypes.bfloat16
    wq = np.array(w_qkv, dtype=np.float32, copy=True)
    wq[:HID] *= SCALE  # fold attention scale into the q projection
    wqkvT = np.ascontiguousarray(wq.T).astype(bf16)  # [256, 1536]
    woutT = np.ascontiguousarray(
        w_out.T.reshape(4, 128, DIM).transpose(1, 0, 2)
    ).astype(bf16)  # [128, 4, 256]
    bias = np.ascontiguousarray(
        b_out.astype(np.float32).reshape(DIM // 128, 128).T
    )  # [128, 2]
    in_maps = []
    for i in range(N_CORES):
        in_maps.append(
            {
                "x": x[i].astype(bf16),
                "wqkvT": wqkvT,
                "woutT": woutT,
                "bias": bias,
            }
        )
    return in_maps


def kernel(x, w_qkv, w_out, b_out, _trace=False):
    nc = _get_nc()
    in_maps = make_in_maps(x, w_qkv, w_out, b_out)
    res = run_bass_kernel_spmd(
        nc,
        in_maps,
        core_ids=list(range(N_CORES)),
        trace=_trace,
        trace_cores=list(range(N_CORES)) if _trace else None,
    )
    out = np.stack([res.results[i]["out"] for i in range(N_CORES)], axis=0)
    if _trace:
        kernel.last_exec_time_ns = res.exec_time_ns
        kernel.last_results = res
    return out


# revision 35
# speedup vs baseline: 1.2042x; 1.2042x over previous
"""Trainium2 Bass kernel for nn_Attention_15539191677265.

Single-head-dim attention block:
    qkv = w_qkv @ x ; per-head scaled dot-product attention over w=2048;
    out = w_out @ attn_out + b_out

Sharding: pure data-parallel over batch (b=8 -> 8 NeuronCores, one batch
element per core). Weights are replicated. No collectives.

Per-core algorithm (transposed-softmax, bf16 matmuls):
  1. q,k projections, then DMA-duplicate each head's 64 d-rows into both
     PE row halves so consecutive sim matmuls alternate row groups.
  2. vT = x.T @ wvT with 64 ones-columns appended: AV psum rows 64..127
     accumulate the softmax normalizer PRE-BROADCAST across partitions.
  3. per (head, i-half 1024, j-tile): sim^T strip as two full-width K=64
     matmuls on alternating PE row-halves (tile_position (0,0)/(64,0)) --
     disjoint rows + different psum banks stream concurrently (~2x column
     rate); exp as ONE [128,1024] op, split across engines: 9/16 j-tiles
     on ScalarE (exact exp), 7/16 on VectorE via the Schraudolph bf16
     bit-trick (x*128/ln2 + magic -> int16 bits == bf16 exp approx,
     ~1.8% rms weight noise, cancels through the shared normalizer);
     AV (M=128, K=128) accumulates over j-tiles in 2-jt batches lagging
     sim by 2-3 j-tiles so the PE never waits on the exp engines; each
     block's last two AVs + normalization are emitted inside the NEXT
     block's first j-tiles (cross-block pipelining) so the final exp
     wait is filled with the next block's sim matmuls. Three strip
     buffers + one AV buffer (8 psum banks exactly): sims wait on
     exp(jt-3), hiding the strip-rotation semaphore latency.
  4. normalize: copy av -> SBUF (ScalarE/VectorE), DMA-shift the
     normalizer rows to partitions 0..63, reciprocal (DVE custom op at
     base partition 0), multiply on GpSimd; odd heads bounce via DMA.
  5. proj: out = woutT.T @ attn_out (K=128 head-pair chunks) + bias.

No max subtraction before exp: scores are ~N(0,1) so exp cannot
overflow in bf16/fp32.
"""

import sys

if "/opt/trn_rl_repo" not in sys.path:
    sys.path.insert(0, "/opt/trn_rl_repo")

import numpy as np
import ml_dtypes

import concourse.bass as bass
import concourse.mybir as mybir
import concourse.tile as tile
from concourse import bacc
from concourse.bass_utils import run_bass_kernel_spmd

BF16 = mybir.dt.bfloat16
F32 = mybir.dt.float32
I16 = mybir.dt.int16
EXP = mybir.ActivationFunctionType.Exp

B, DIM, W = 8, 256, 2048
HEADS, DH = 8, 64
HID = HEADS * DH  # 512
SCALE = DH ** (-0.5)
N_CORES = 8

NJT = W // 128  # 16 j-tiles per head
NCT = DIM // 128  # 2 contraction chunks over channels
IH = 1024  # i-half width
NIH = W // IH

# j-tiles whose exp runs on VectorE via Schraudolph (rest: exact on ScalarE)
DVE_JTS = frozenset({1, 3, 5, 8, 10, 12, 14})
SCH_MUL = 128.0 / float(np.log(2.0))  # 184.6645
SCH_ADD = 127.0 * 128.0 - 7.4  # PWL-centering constant (round-to-nearest)


def build_kernel():
    nc = bacc.Bacc(None, target_bir_lowering=False)

    x_d = nc.dram_tensor("x", [DIM, W], BF16, kind="ExternalInput")
    wqkvT_d = nc.dram_tensor("wqkvT", [DIM, 3 * HID], BF16, kind="ExternalInput")
    woutT_d = nc.dram_tensor("woutT", [128, 4, DIM], BF16, kind="ExternalInput")
    bias_d = nc.dram_tensor("bias", [128, DIM // 128], F32, kind="ExternalInput")
    out_d = nc.dram_tensor("out", [DIM, W], F32, kind="ExternalOutput")

    with tile.TileContext(nc) as tc:
        with tc.tile_pool(name="pers", bufs=1) as pers:
            x_sb = pers.tile([128, NCT, W], BF16)
            wq_sb = pers.tile([128, NCT, 3 * HID], BF16)
            wo_sb = pers.tile([128, 4, DIM], BF16)
            bias_sb = pers.tile([128, DIM // 128], F32)
            # [128 = head d-rows duplicated in both halves, head, w]
            q_sb = pers.tile([128, HEADS, W], BF16)
            k_sb = pers.tile([128, HEADS, W], BF16)
            vt_sb = pers.tile([128, NJT, HEADS, 128], BF16)
            attout_sb = [
                pers.tile([128, W], BF16, name=f"attout_{kc}", tag=f"attout{kc}")
                for kc in range(4)
            ]
            out_sb = pers.tile([128, NCT, W], F32)

            # coalesced input loads: one DMA per contraction half removes
            # the per-chunk descriptor-issue gaps on the serial SP queue
            xr = x_d[:].rearrange("(ct p) w -> p ct w", p=128)
            wqr = wqkvT_d[:].rearrange("(ct p) o -> p ct o", p=128)
            for ct in range(NCT):
                nc.sync.dma_start(out=wq_sb[:, ct, :], in_=wqr[:, ct, :])
            for ct in range(NCT):
                nc.sync.dma_start(out=x_sb[:, ct, :], in_=xr[:, ct, :])
            nc.sync.dma_start(out=wo_sb[:], in_=woutT_d[:])
            nc.sync.dma_start(out=bias_sb[:], in_=bias_d[:])

            # vT cols 64..127: ALL ones -> AV rows 64..127 hold the softmax
            # normalizer replicated across 64 partitions (pre-broadcast)
            nc.vector.memset(vt_sb[:, :, :, DH:128], 1.0)
            # warm the ACT exp table while qkv matmuls run
            warm = pers.tile([1, 1], F32)
            nc.vector.memset(warm[:], 0.0)
            nc.scalar.activation(out=warm[:], in_=warm[:], func=EXP)

            # ---- phase 1: v first (attention needs all of vT), then q,k
            with tc.tile_pool(name="qkv_ps", bufs=4, space="PSUM") as qkv_ps:

                def emit_qk_otile(ot):
                    for dst, base in ((q_sb, 0), (k_sb, HID)):
                        for ph in range(4):
                            po = ph * 512
                            ps = qkv_ps.tile(
                                [128, 512], F32, name=f"qk_{ot}_{base}_{ph}", tag="qk"
                            )
                            for ct in range(NCT):
                                nc.tensor.matmul(
                                    ps[:],
                                    lhsT=wq_sb[:, ct, base + ot * 128 : base + (ot + 1) * 128],
                                    rhs=x_sb[:, ct, po : po + 512],
                                    start=(ct == 0),
                                    stop=(ct == NCT - 1),
                                )
                            if ph % 2 == 0:
                                nc.vector.tensor_copy(
                                    out=dst[0:64, 2 * ot, po : po + 512],
                                    in_=ps[0:64, :],
                                )
                                nc.scalar.copy(
                                    out=dst[64:128, 2 * ot + 1, po : po + 512],
                                    in_=ps[64:128, :],
                                )
                            else:
                                nc.scalar.copy(
                                    out=dst[0:64, 2 * ot, po : po + 512],
                                    in_=ps[0:64, :],
                                )
                                nc.vector.tensor_copy(
                                    out=dst[64:128, 2 * ot + 1, po : po + 512],
                                    in_=ps[64:128, :],
                                )
                        # partition-duplicating DMAs (overlap with matmuls)
                        nc.sync.dma_start(
                            out=dst[64:128, 2 * ot, :], in_=dst[0:64, 2 * ot, :]
                        )
                        nc.sync.dma_start(
                            out=dst[0:64, 2 * ot + 1, :], in_=dst[64:128, 2 * ot + 1, :]
                        )

                for jt in range(NJT):
                    ps = qkv_ps.tile([128, HID], F32, name=f"vt_{jt}", tag="vt")
                    for ct in range(NCT):
                        nc.tensor.matmul(
                            ps[:],
                            lhsT=x_sb[:, ct, jt * 128 : (jt + 1) * 128],
                            rhs=wq_sb[:, ct, 2 * HID : 3 * HID],
                            start=(ct == 0),
                            stop=(ct == NCT - 1),
                        )
                    if jt % 2 == 0:
                        nc.vector.tensor_copy(
                            out=vt_sb[:, jt, :, 0:DH],
                            in_=ps[:].rearrange("p (h d) -> p h d", h=HEADS),
                        )
                    else:
                        nc.scalar.copy(
                            out=vt_sb[:, jt, :, 0:DH],
                            in_=ps[:].rearrange("p (h d) -> p h d", h=HEADS),
                        )

                for ot in range(4):
                    emit_qk_otile(ot)

            # ---- phase 3: attention ----
            with (
                tc.tile_pool(name="strip_ps", bufs=3, space="PSUM") as strip_ps,
                tc.tile_pool(name="av_ps", bufs=1, space="PSUM") as av_ps,
                tc.tile_pool(name="exp_sb", bufs=6) as exp_pool,
                tc.tile_pool(name="norm_sb", bufs=2) as norm_pool,
            ):
                def emit_av(av, h, jt, es_tiles):
                    es = es_tiles[jt]
                    for c in range(IH // 512):
                        co = c * 512
                        nc.tensor.matmul(
                            av[:, co : co + 512],
                            lhsT=vt_sb[:, jt, h, :],
                            rhs=es[:, co : co + 512],
                            start=(jt == 0),
                            stop=(jt == NJT - 1),
                        )

                def norm_chain(av, h, io):
                    ncp = norm_pool.tile([128, IH], F32, tag="ncp")
                    if h % 2 == 0:
                        nc.scalar.copy(out=ncp[:], in_=av[:])
                    else:
                        nc.vector.tensor_copy(out=ncp[:], in_=av[:])
                    nsh = norm_pool.tile([DH, IH], F32, tag="nsh")
                    nc.sync.dma_start(out=nsh[:], in_=ncp[DH:128, :])
                    # custom-DVE recip requires base partition 0
                    nrm = norm_pool.tile([DH, IH], F32, tag="nrm")
                    nc.vector.reciprocal_approx_fast(out=nrm[:], in_=nsh[:])
                    # SBUF-only multiply on the otherwise-idle GpSimd
                    if h % 2 == 0:
                        nc.gpsimd.tensor_mul(
                            out=attout_sb[h // 2][0:DH, io : io + IH],
                            in0=ncp[0:DH, :],
                            in1=nrm[:],
                        )
                    else:
                        # odd heads land on partitions 64..127: bounce
                        atmp = norm_pool.tile([DH, IH], BF16, tag="atmp")
                        nc.gpsimd.tensor_mul(
                            out=atmp[:], in0=ncp[0:DH, :], in1=nrm[:]
                        )
                        nc.sync.dma_start(
                            out=attout_sb[h // 2][DH:128, io : io + IH],
                            in_=atmp[:],
                        )

                # odd heads (bounce DMA) early; cross-block pipelining: each
                # block's last two AVs + norm chain are emitted inside the
                # NEXT block's first j-tiles so the PE fills the final exp
                # wait with the next block's sim matmuls
                pending = None
                for h in (1, 3, 5, 7, 0, 2, 4, 6):
                    for ih in range(NIH):
                        io = ih * IH
                        av = av_ps.tile([128, IH], F32, name=f"av_{h}_{ih}", tag="av")
                        es_tiles = {}
                        for jt in range(NJT):
                            strip = strip_ps.tile(
                                [128, IH], F32, name=f"st_{h}_{ih}_{jt}", tag="st"
                            )
                            for c in range(IH // 512):
                                co = c * 512
                                # full-width K=64 matmuls on alternating PE
                                # row-halves: adjacent ops use disjoint rows
                                # and different psum banks -> concurrent
                                rg = 64 * (c % 2)
                                nc.tensor.matmul(
                                    strip[:, co : co + 512],
                                    lhsT=k_sb[rg : rg + 64, h, jt * 128 : (jt + 1) * 128],
                                    rhs=q_sb[rg : rg + 64, h, io + co : io + co + 512],
                                    start=True,
                                    stop=True,
                                    tile_position=(rg, 0),
                                )
                            es = exp_pool.tile([128, IH], BF16, name=f"es_{jt}", tag="es")
                            es_tiles[jt] = es
                            if jt in DVE_JTS:
                                nc.vector.tensor_scalar(
                                    out=es[:].bitcast(I16),
                                    in0=strip[:],
                                    scalar1=SCH_MUL,
                                    scalar2=SCH_ADD,
                                    op0=mybir.AluOpType.mult,
                                    op1=mybir.AluOpType.add,
                                )
                            else:
                                nc.scalar.activation(out=es[:], in_=strip[:], func=EXP)
                            if jt == 0:
                                if pending is not None:
                                    emit_av(pending[0], pending[1], NJT - 2, pending[3])
                            elif jt == 1:
                                if pending is not None:
                                    emit_av(pending[0], pending[1], NJT - 1, pending[3])
                                    norm_chain(pending[0], pending[1], pending[2])
                            elif jt % 2 == 1:
                                # AVs in 2-jt batches: halves the PE's
                                # 64-row/128-row tile-config switches
                                emit_av(av, h, jt - 3, es_tiles)
                                emit_av(av, h, jt - 2, es_tiles)
                        pending = (av, h, io, es_tiles)
                emit_av(pending[0], pending[1], NJT - 2, pending[3])
                emit_av(pending[0], pending[1], NJT - 1, pending[3])
                norm_chain(pending[0], pending[1], pending[2])

            # ---- phase 4: output projection + bias ----
            outr = out_d[:].rearrange("(ct p) w -> p ct w", p=128)
            with tc.tile_pool(name="proj_ps", bufs=8, space="PSUM") as proj_ps:
                for ot in range(NCT):
                    for wh in range(4):
                        wo = wh * 512
                        ps = proj_ps.tile([128, 512], F32, name=f"pj_{ot}_{wh}", tag="pj")
                        for kc in range(4):
                            nc.tensor.matmul(
                                ps[:],
                                lhsT=wo_sb[:, kc, ot * 128 : (ot + 1) * 128],
                                rhs=attout_sb[kc][:, wo : wo + 512],
                                start=(kc == 0),
                                stop=(kc == 3),
                            )
                        nc.vector.tensor_scalar_add(
                            out=out_sb[:, ot, wo : wo + 512],
                            in0=ps[:],
                            scalar1=bias_sb[:, ot : ot + 1],
                        )
                        nc.sync.dma_start(
                            out=outr[:, ot, wo : wo + 512],
                            in_=out_sb[:, ot, wo : wo + 512],
                        )

    nc.compile()
    return nc


_NC_CACHE = None


def _get_nc():
    global _NC_CACHE
    if _NC_CACHE is None:
        _NC_CACHE = build_kernel()
    return _NC_CACHE


def make_in_maps(x, w_qkv, w_out, b_out):
    bf16 = ml_dtypes.bfloat16
    wq = np.array(w_qkv, dtype=np.float32, copy=True)
    wq[:HID] *= SCALE  # fold attention scale into the q projection
    wqkvT = np.ascontiguousarray(wq.T).astype(bf16)  # [256, 1536]
    woutT = np.ascontiguousarray(
        w_out.T.reshape(4, 128, DIM).transpose(1, 0, 2)
    ).astype(bf16)  # [128, 4, 256]
    bias = np.ascontiguousarray(
        b_out.astype(np.float32).reshape(DIM // 128, 128).T
    )  # [128, 2]
    in_maps = []
    for i in range(N_CORES):
        in_maps.append(
            {
                "x": x[i].astype(bf16),
                "wqkvT": wqkvT,
                "woutT": woutT,
                "bias": bias,
            }
        )
    return in_maps


def kernel(x, w_qkv, w_out, b_out, _trace=False):
    nc = _get_nc()
    in_maps = make_in_maps(x, w_qkv, w_out, b_out)
    res = run_bass_kernel_spmd(
        nc,
        in_maps,
        core_ids=list(range(N_CORES)),
        trace=_trace,
        trace_cores=list(range(N_CORES)) if _trace else None,
    )
    out = np.stack([res.results[i]["out"] for i in range(N_CORES)], axis=0)
    if _trace:
        kernel.last_exec_time_ns = res.exec_time_ns
        kernel.last_results = res
    return out
